# revision 1
# baseline (speedup 1.0000x reference)
"""Gammatone filterbank (4 cascaded complex one-pole IIR sections) on TRN2.

Algorithm (per waveform; all 128 bands in parallel on the 128 partitions):
  The complex recurrence s[t] = c*s[t-1] + u[t] with c = lam*e^{i*beta} is
  derotated per chunk: with sh[j] = s[t0+j]*e^{-i*j*beta} the recurrence
  becomes sh[j] = lam*sh[j-1] + u[t0+j]*e^{-i*j*beta} with REAL lam, so
  re/im decouple into independent real first-order scans on the DVE
  (tensor_tensor_scan).  The 4-stage cascade = 8 real scans per chunk.
  Mod/demod tables are chunk-local ([C, K], static in SBUF); each carries
  sqrt(factor) so the pair applies the stage-1 gain exactly once:
    Ar =  sqrt(f)*cos(j*b)*x,   Ai = -sqrt(f)*sin(j*b)*x
    (4x) Ar <- scan(lam, Ar),   Ai <- scan(lam, Ai)
    out = sqrt(f)*cos(j*b)*Yr + (-sqrt(f)*sin(j*b))*Yi = f*Re[cascade4(c,x)]
  Scan state carries across chunks via the scan `initial` operand; the
  carried complex state is rotated by e^{+i*K*beta} (per-channel constants
  cK/sK) to bridge chunk-local phase frames.

Engine split per chunk: DVE runs the 8 scans + the tiny batched state
rotation; GPSIMD runs the x partition-broadcast, the 2 modulation / 2
demodulation multiplies and the final add; the Scalar engine copies each
scan's last column into the state tile.  The broadcast+modulation for
chunk p+1 is emitted before chunk p's demodulation so GPSIMD feeds the
DVE ahead of time (software pipeline).  DMA traffic per core is just the
16 MB output + 128 KB input + one-time constants.

Sharding: batch-parallel SPMD, one waveform per NeuronCore (8 cores, B=8).
Output is [C, T] per core; the host transposes/stacks to [B, T, C].
"""

import sys

import numpy as np

for _p in ("/opt/trn_rl_repo",):
    if _p not in sys.path:
        sys.path.insert(0, _p)

import concourse.bass as bass  # noqa: F401
import concourse.mybir as mybir
from concourse.bacc import Bacc
from concourse.bass_utils import run_bass_kernel_spmd
from concourse.tile import TileContext

B = 8
T = 32000
C = 128
K = 2000          # time-chunk length (columns per DVE op)
NCHUNK = T // K
F32 = mybir.dt.float32
MULT = mybir.AluOpType.mult
ADD = mybir.AluOpType.add
SUB = mybir.AluOpType.subtract


def build_bass(t_len=T, k=K):
    nchunk = t_len // k
    assert nchunk * k == t_len
    nc = Bacc()
    x = nc.declare_dram_parameter("x", [1, t_len], F32, isOutput=False)
    mcl = nc.declare_dram_parameter("mcl", [C, k], F32, isOutput=False)
    msl = nc.declare_dram_parameter("msl", [C, k], F32, isOutput=False)
    lamt = nc.declare_dram_parameter("lamt", [C, k], F32, isOutput=False)
    ckp = nc.declare_dram_parameter("ck", [C, 1], F32, isOutput=False)
    skp = nc.declare_dram_parameter("sk", [C, 1], F32, isOutput=False)
    out = nc.declare_dram_parameter("out", [C, t_len], F32, isOutput=True)

    with TileContext(nc) as tc:
        with (
            tc.tile_pool(name="consts", bufs=1) as consts,
            tc.tile_pool(name="dmat", bufs=2) as dmat,
            tc.tile_pool(name="work", bufs=1) as work,
            tc.tile_pool(name="states", bufs=2) as stp,
        ):
            tabc = consts.tile([C, k], F32, tag="mcl", name="tabc")
            tabs = consts.tile([C, k], F32, tag="msl", name="tabs")
            lam_t = consts.tile([C, k], F32, tag="lam", name="lam_t")
            ck = consts.tile([C, 1], F32, tag="ck", name="ck")
            sk = consts.tile([C, 1], F32, tag="sk", name="sk")
            nc.sync.dma_start(out=tabc[:], in_=mcl[:])
            nc.sync.dma_start(out=tabs[:], in_=msl[:])
            nc.sync.dma_start(out=lam_t[:], in_=lamt[:])
            nc.sync.dma_start(out=ck[:], in_=ckp[:])
            nc.sync.dma_start(out=sk[:], in_=skp[:])

            def bcast_mod(p):
                """DMA x chunk p, broadcast it, modulate (GPSIMD)."""
                t0 = p * k
                xrow = dmat.tile([1, k], F32, tag="xrow", name="xrow")
                nc.sync.dma_start(out=xrow[:], in_=x[0:1, t0:t0 + k])
                xb = dmat.tile([C, k], F32, tag="xb", name="xb")
                nc.gpsimd.partition_broadcast(xb[:], xrow[:])
                mr = dmat.tile([C, k], F32, tag="Mr", name="mr")
                mi = dmat.tile([C, k], F32, tag="Mi", name="mi")
                nc.gpsimd.tensor_tensor(mr[:], tabc[:], xb[:], MULT)
                nc.gpsimd.tensor_tensor(mi[:], tabs[:], xb[:], MULT)
                return mr, mi

            # rotated initial states for the current chunk:
            # cols 0:4 = re(stage1..4), cols 4:8 = im(stage1..4)
            st_rot = stp.tile([C, 8], F32, tag="st_rot", name="st0")
            nc.vector.memset(st_rot[:], 0.0)

            mod_tiles = bcast_mod(0)
            for p in range(nchunk):
                t0 = p * k
                last = p == nchunk - 1
                mr, mi = mod_tiles

                # 4 cascaded one-pole stages = 8 real scans (DVE);
                # ACT saves each scan's last column as raw carried state.
                st_raw = stp.tile([C, 8], F32, tag="st_raw", name="st_raw")
                cur_r, cur_i = mr, mi
                for stage in range(4):
                    ab = stage % 2 == 0
                    nr = work.tile([C, k], F32, tag="Ar" if ab else "Br",
                                   name="nr", bufs=1 if ab else 2)
                    ni = work.tile([C, k], F32, tag="Ai" if ab else "Bi",
                                   name="ni", bufs=1 if ab else 2)
                    nc.vector.tensor_tensor_scan(
                        nr[:], lam_t[:], cur_r[:],
                        st_rot[:, stage:stage + 1], MULT, ADD)
                    nc.vector.tensor_tensor_scan(
                        ni[:], lam_t[:], cur_i[:],
                        st_rot[:, 4 + stage:5 + stage], MULT, ADD)
                    if not last:
                        nc.scalar.copy(
                            out=st_raw[:, stage:stage + 1],
                            in_=nr[:, k - 1:k])
                        nc.scalar.copy(
                            out=st_raw[:, 4 + stage:5 + stage],
                            in_=ni[:, k - 1:k])
                    cur_r, cur_i = nr, ni

                if not last:
                    # rotate carried state by e^{+i*K*beta} (batched, DVE):
                    # new_re = re*cK - im*sK ; new_im = im*cK + re*sK
                    tmp = stp.tile([C, 8], F32, tag="st_tmp", name="tmp")
                    nxt_rot = stp.tile([C, 8], F32, tag="st_rot", name="nxt")
                    nc.vector.tensor_scalar(
                        tmp[:, 0:4], st_raw[:, 4:8], sk[:], None, MULT)
                    nc.vector.tensor_scalar(
                        tmp[:, 4:8], st_raw[:, 0:4], sk[:], None, MULT)
                    nc.vector.scalar_tensor_tensor(
                        nxt_rot[:, 0:4], st_raw[:, 0:4], ck[:], tmp[:, 0:4],
                        MULT, SUB)
                    nc.vector.scalar_tensor_tensor(
                        nxt_rot[:, 4:8], st_raw[:, 4:8], ck[:], tmp[:, 4:8],
                        MULT, ADD)
                    st_rot = nxt_rot
                    # feed GPSIMD chunk p+1's inputs before demod of chunk p
                    mod_tiles = bcast_mod(p + 1)

                # demodulate (GPSIMD): z = tabc*Yr + tabs*Yi
                zr = work.tile([C, k], F32, tag="Dr", name="zr")
                zi = work.tile([C, k], F32, tag="Di", name="zi")
                z = dmat.tile([C, k], F32, tag="z", name="z")
                nc.gpsimd.tensor_tensor(zr[:], tabc[:], cur_r[:], MULT)
                nc.gpsimd.tensor_tensor(zi[:], tabs[:], cur_i[:], MULT)
                nc.gpsimd.tensor_tensor(z[:], zr[:], zi[:], ADD)

                nc.sync.dma_start(out=out[:, t0:t0 + k], in_=z[:])
    nc.finalize()
    return nc


def make_tables(coef_re, coef_im, factor, t_len=T, k=K):
    cr = np.asarray(coef_re, np.float64)
    ci = np.asarray(coef_im, np.float64)
    f = np.asarray(factor, np.float64)
    lam = np.hypot(cr, ci)
    beta = np.arctan2(ci, cr)
    sf = np.sqrt(f)
    j = np.arange(k, dtype=np.float64)
    ph = j[None, :] * beta[:, None]
    mcl = (sf[:, None] * np.cos(ph)).astype(np.float32)      # [C, K]
    msl = (-sf[:, None] * np.sin(ph)).astype(np.float32)     # [C, K]
    lam_tile = np.broadcast_to(lam.astype(np.float32)[:, None], (C, k)).copy()
    kb = k * beta
    ck = np.cos(kb).astype(np.float32)[:, None]              # [C, 1]
    sk = np.sin(kb).astype(np.float32)[:, None]              # [C, 1]
    return mcl, msl, lam_tile, ck, sk


_CACHED_NC = None


def kernel(inp, coef_re, coef_im, factor):
    global _CACHED_NC
    inp = np.ascontiguousarray(np.asarray(inp, np.float32))
    assert inp.shape == (B, T)
    mcl, msl, lam_tile, ck, sk = make_tables(coef_re, coef_im, factor)

    if _CACHED_NC is None:
        _CACHED_NC = build_bass()
    nc = _CACHED_NC

    in_maps = [
        {"x": inp[i:i + 1, :], "mcl": mcl, "msl": msl, "lamt": lam_tile,
         "ck": ck, "sk": sk}
        for i in range(B)
    ]
    res = run_bass_kernel_spmd(nc, in_maps, core_ids=list(range(B)))
    out = np.stack([np.asarray(res.results[i]["out"]).T for i in range(B)])
    return np.ascontiguousarray(out.astype(np.float32))

